# revision 7
# baseline (speedup 1.0000x reference)
import sys
from contextlib import ExitStack

import numpy as np

if "/opt/trn_rl_repo" not in sys.path:
    sys.path.insert(0, "/opt/trn_rl_repo")

from concourse import bacc, bass, tile
from concourse.bass_utils import run_bass_kernel_spmd

mybir = bass.mybir
F32 = mybir.dt.float32
AF = mybir.ActivationFunctionType

B, T, D, H = 16, 256, 128, 256
NC, NB = 8, 2
G3 = 3 * H

TRACE = False
LAST_EXEC_NS = None
_TIME_STATE = {}


def _build_program():
    nc = bacc.Bacc("TRN2", target_bir_lowering=False, debug=False, num_devices=NC)

    shapes = {
        "xT": [128, T, NB],
        "xgT": [128, T, NB],
        "wfeT": [128, G3],
        "wfdT": [128, G3],
        "whhTe": [128, 2, G3],
        "whhTd": [128, 2, G3],
        "biasE": [128, 6],
        "biasD": [128, 6],
        "wkT": [128, 2, H],
        "wqT": [128, 2, H],
        "vT": [128, 2],
        "mbT": [128, NB, 2, T],
    }
    dram_in = {
        name: nc.dram_tensor(name, shp, F32, kind="ExternalInput")
        for name, shp in shapes.items()
    }
    out_dram = nc.dram_tensor("outT", [128, NB, 2, T], F32, kind="ExternalOutput")

    with tile.TileContext(nc) as tc, ExitStack() as ctx:
        wp = ctx.enter_context(tc.tile_pool(name="wp", bufs=1))
        sp = ctx.enter_context(tc.tile_pool(name="state", bufs=1))
        gp = ctx.enter_context(tc.tile_pool(name="gates", bufs=3))

        sb = {}
        for name, shp in shapes.items():
            t_ = wp.tile(shp, F32, name=f"{name}_sb")
            nc.sync.dma_start(t_[:], dram_in[name][:])
            sb[name] = t_

        enc_h = sp.tile([128, T + 1, 2, NB], F32, name="enc_h")
        dec_h = sp.tile([128, T + 1, 2, NB], F32, name="dec_h")
        gi_e = sp.tile([128, T, 3, 2, NB], F32, name="gi_e")
        gi_d = sp.tile([128, T, 3, 2, NB], F32, name="gi_d")

        # input projections: gi = x @ (Wih @ W_enc).T + bias, all t at once
        with tc.tile_pool(name="ps_gi", bufs=2, space="PSUM") as ps_gi:
            for gi, wf, bias, x in (
                (gi_e, sb["wfeT"], sb["biasE"], sb["xT"]),
                (gi_d, sb["wfdT"], sb["biasD"], sb["xgT"]),
            ):
                for g in range(3):
                    for o in range(2):
                        j = g * 2 + o
                        ps = ps_gi.tile([128, T, NB], F32, name="gi_ps")
                        nc.tensor.matmul(
                            ps[:],
                            wf[:, j * 128 : (j + 1) * 128],
                            x[:],
                            start=True,
                            stop=True,
                        )
                        nc.scalar.activation(
                            gi[:, :, g, o, :], ps[:], AF.Identity,
                            bias=bias[:, j : j + 1],
                        )

        nc.gpsimd.memset(enc_h[:, 0, :, :], 0.0)

        def gru_steps(h_all, gi, whhT):
            with tc.tile_pool(name="ps_gh", bufs=2, space="PSUM") as ps_gh:
                for t in range(T):
                    gh = ps_gh.tile([128, 3, 2, NB], F32, name="gh")
                    hp = h_all[:, t, :, :]
                    # PE order r, n, z keeps the tanh path short
                    for g in (0, 2, 1):
                        for o in range(2):
                            for kc in range(2):
                                gd = g * H + o * 128
                                nc.tensor.matmul(
                                    gh[:, g, o, :],
                                    whhT[:, kc, gd : gd + 128],
                                    h_all[:, t, kc, :],
                                    start=(kc == 0),
                                    stop=(kc == 1),
                                )
                        if g == 0:
                            rs = gp.tile([128, 2, NB], F32, name="rs")
                            nc.vector.tensor_add(rs[:], gh[:, 0, :, :], gi[:, t, 0, :, :])
                            r = gp.tile([128, 2, NB], F32, name="r")
                            nc.scalar.activation(r[:], rs[:], AF.Sigmoid)
                        elif g == 2:
                            nm = gp.tile([128, 2, NB], F32, name="nm")
                            nc.vector.tensor_mul(nm[:], r[:], gh[:, 2, :, :])
                            ns = gp.tile([128, 2, NB], F32, name="ns")
                            nc.vector.tensor_add(ns[:], nm[:], gi[:, t, 2, :, :])
                            n = gp.tile([128, 2, NB], F32, name="n")
                            nc.scalar.activation(n[:], ns[:], AF.Tanh)
                        else:
                            zs = gp.tile([128, 2, NB], F32, name="zs")
                            nc.vector.tensor_add(zs[:], gh[:, 1, :, :], gi[:, t, 1, :, :])
                            z = gp.tile([128, 2, NB], F32, name="z")
                            nc.scalar.activation(z[:], zs[:], AF.Sigmoid)
                    hn = gp.tile([128, 2, NB], F32, name="hn")
                    nc.vector.tensor_sub(hn[:], hp, n[:])
                    zhn = gp.tile([128, 2, NB], F32, name="zhn")
                    nc.vector.tensor_mul(zhn[:], z[:], hn[:])
                    nc.vector.tensor_add(h_all[:, t + 1, :, :], n[:], zhn[:])

        gru_steps(enc_h, gi_e, sb["whhTe"])
        nc.vector.tensor_copy(dec_h[:, 0, :, :], enc_h[:, T, :, :])
        gru_steps(dec_h, gi_d, sb["whhTd"])

        # attention: scores[b,t,s] = sum_a v[a] * tanh(q[b,t,a] + k[b,s,a])
        k_sb = sp.tile([128, NB, 2, T], F32, name="k_sb")
        q_sb = sp.tile([128, NB, 2, T], F32, name="q_sb")
        with tc.tile_pool(name="ps_at", bufs=2, space="PSUM") as ps_at:
            for dst, wT, h_all in ((k_sb, sb["wkT"], enc_h), (q_sb, sb["wqT"], dec_h)):
                for bi in range(NB):
                    for ac in range(2):
                        ps = ps_at.tile([128, T], F32, name="proj_ps")
                        for kc in range(2):
                            nc.tensor.matmul(
                                ps[:],
                                wT[:, kc, ac * 128 : (ac + 1) * 128],
                                h_all[:, 1 : T + 1, kc, bi],
                                start=(kc == 0),
                                stop=(kc == 1),
                            )
                        nc.scalar.activation(dst[:, bi, ac, :], ps[:], AF.Identity, bias=0.0)

        TB = 4
        with (
            tc.tile_pool(name="ps_sc", bufs=1, space="PSUM") as ps_sc,
            tc.tile_pool(name="attn", bufs=2) as ap_,
        ):
            for bi in range(NB):
                scores = [
                    ps_sc.tile([128, T], F32, name=f"score_{bi}_{s}") for s in range(2)
                ]
                for tb in range(0, T, TB):
                    kq = ap_.tile([128, 2, TB, T], F32, name="kq")
                    for ti in range(TB):
                        for ac in range(2):
                            nc.vector.tensor_scalar_add(
                                kq[:, ac, ti, :],
                                k_sb[:, bi, ac, :],
                                q_sb[:, bi, ac, tb + ti : tb + ti + 1],
                            )
                    th = ap_.tile([128, 2, TB, T], F32, name="th")
                    nc.scalar.activation(th[:], kq[:], AF.Tanh)
                    for ti in range(TB):
                        for s in range(2):
                            for ac in range(2):
                                nc.tensor.matmul(
                                    scores[s][:, tb + ti : tb + ti + 1],
                                    th[:, ac, ti, s * 128 : (s + 1) * 128],
                                    sb["vT"][:, ac : ac + 1],
                                    start=(ac == 0),
                                    stop=(ac == 1),
                                )
                for s in range(2):
                    lg = ap_.tile([128, T], F32, name="lg")
                    nc.vector.tensor_add(lg[:], scores[s][:], sb["mbT"][:, bi, s, :])
                    nc.sync.dma_start(out_dram[:, bi, s, :], lg[:])

    if not nc.is_finalized():
        nc.finalize()
    return nc, list(shapes.keys())


def kernel(**inputs):
    global LAST_EXEC_NS
    x = np.ascontiguousarray(np.asarray(inputs["inputs"], dtype=np.float32))
    targets = np.asarray(inputs["targets"]).astype(np.int64)
    f64 = np.float64

    def fuse(Wih, bih, bhh, W_enc, b_enc):
        Wf = (Wih.astype(f64) @ W_enc.astype(f64)).astype(np.float32)
        bf = (
            Wih.astype(f64) @ b_enc.astype(f64)
            + bih.astype(f64)
            + bhh.astype(f64)
        ).astype(np.float32)
        return Wf, bf

    W_enc = np.asarray(inputs["W_enc"], dtype=np.float32)
    b_enc = np.asarray(inputs["b_enc"], dtype=np.float32)
    Wfe, bfe = fuse(
        np.asarray(inputs["enc_Wih"], dtype=np.float32),
        np.asarray(inputs["enc_bih"], dtype=np.float32),
        np.asarray(inputs["enc_bhh"], dtype=np.float32),
        W_enc, b_enc,
    )
    Wfd, bfd = fuse(
        np.asarray(inputs["dec_Wih"], dtype=np.float32),
        np.asarray(inputs["dec_bih"], dtype=np.float32),
        np.asarray(inputs["dec_bhh"], dtype=np.float32),
        W_enc, b_enc,
    )

    def whhT_layout(Whh):
        return np.ascontiguousarray(
            np.asarray(Whh, dtype=np.float32).T.reshape(2, 128, G3).transpose(1, 0, 2)
        )

    def hT_layout(Wm):  # [H, H] -> [128, 2, H]
        return np.ascontiguousarray(
            np.asarray(Wm, dtype=np.float32).T.reshape(2, 128, H).transpose(1, 0, 2)
        )

    whhTe = whhT_layout(inputs["enc_Whh"])
    whhTd = whhT_layout(inputs["dec_Whh"])
    wkT = hT_layout(inputs["Wk"])
    wqT = hT_layout(inputs["Wq"])
    vT = np.ascontiguousarray(
        np.asarray(inputs["v"], dtype=np.float32).reshape(2, 128).T
    )
    wfeT = np.ascontiguousarray(Wfe.T)
    wfdT = np.ascontiguousarray(Wfd.T)
    biasE = np.ascontiguousarray(bfe.reshape(6, 128).T)
    biasD = np.ascontiguousarray(bfd.reshape(6, 128).T)

    dec_idx = np.roll(targets, 1, axis=1)
    xg = np.take_along_axis(x, dec_idx[:, :, None], axis=1)

    onehot = (targets[:, :, None] == np.arange(T)[None, None, :]).astype(np.int32)
    prev = np.cumsum(onehot, axis=1) - onehot
    mb = np.where(prev == 0, np.float32(0.0), np.float32(-1e9)).astype(np.float32)

    nc, in_names = _build_program()

    in_maps = []
    for c in range(NC):
        bs = slice(c * NB, (c + 1) * NB)
        xc = np.ascontiguousarray(x[bs].transpose(2, 1, 0))          # [128, T, NB]
        xgc = np.ascontiguousarray(xg[bs].transpose(2, 1, 0))
        mbc = np.ascontiguousarray(
            mb[bs].transpose(0, 2, 1).reshape(NB, 2, 128, T).transpose(2, 0, 1, 3)
        )                                                            # [128, NB, 2, T]
        in_maps.append({
            "xT": xc, "xgT": xgc,
            "wfeT": wfeT, "wfdT": wfdT,
            "whhTe": whhTe, "whhTd": whhTd,
            "biasE": biasE, "biasD": biasD,
            "wkT": wkT, "wqT": wqT, "vT": vT,
            "mbT": mbc,
        })

    br = run_bass_kernel_spmd(nc, in_maps, list(range(NC)), trace=TRACE)
    if TRACE:
        LAST_EXEC_NS = br.exec_time_ns
    _TIME_STATE["nc"] = nc
    _TIME_STATE["in_maps"] = in_maps

    logits = np.empty((B, T, T), dtype=np.float32)
    for c in range(NC):
        outT = br.results[c]["outT"]                                 # [128, NB, 2, T]
        logits[c * NB : (c + 1) * NB] = outT.transpose(1, 3, 2, 0).reshape(NB, T, T)
    return logits


def _make_pjrt_fn(nc, in_maps):
    import jax
    from concourse import bass2jax

    bass2jax.install_neuronx_cc_hook()
    partition_name = nc.partition_id_tensor.name if nc.partition_id_tensor else None
    in_names, out_names, out_avals, zero_outs = [], [], [], []
    for alloc in nc.m.functions[0].allocations:
        if not isinstance(alloc, bass.mybir.MemoryLocationSet):
            continue
        name = alloc.memorylocations[0].name
        if alloc.kind == "ExternalInput":
            if name != partition_name:
                in_names.append(name)
        elif alloc.kind == "ExternalOutput":
            shape = tuple(alloc.tensor_shape)
            dtype = bass.mybir.dt.np(alloc.dtype)
            out_avals.append(jax.core.ShapedArray(shape, dtype))
            out_names.append(name)
            zero_outs.append(np.zeros(shape, dtype))
    n_params = len(in_names)
    all_in_names = in_names + out_names
    if partition_name is not None:
        all_in_names = all_in_names + [partition_name]

    def _body(*args):
        operands = list(args)
        if partition_name is not None:
            operands.append(bass2jax.partition_id_tensor())
        return tuple(
            bass2jax._bass_exec_p.bind(
                *operands,
                out_avals=tuple(out_avals),
                in_names=tuple(all_in_names),
                out_names=tuple(out_names),
                lowering_input_output_aliases=(),
                sim_require_finite=True,
                sim_require_nnan=True,
                nc=nc,
            )
        )

    n_cores = len(in_maps)
    devices = jax.devices()[:n_cores]
    mesh = bass2jax.Mesh(np.asarray(devices), ("core",))
    P = bass2jax.PartitionSpec
    f = jax.jit(
        bass2jax.shard_map(
            _body,
            mesh=mesh,
            in_specs=(P("core"),) * (n_params + len(out_names)),
            out_specs=(P("core"),) * len(out_names),
            check_rep=False,
        ),
        keep_unused=True,
    )
    sharding = jax.sharding.NamedSharding(mesh, P("core"))
    dev_args = []
    for i, name in enumerate(in_names):
        g = np.concatenate([np.asarray(m[name]) for m in in_maps], axis=0)
        dev_args.append(jax.device_put(g, sharding))
    for z in zero_outs:
        g = np.concatenate([z] * n_cores, axis=0)
        dev_args.append(jax.device_put(g, sharding))
    return f, dev_args


def _time_fn(f, dev_args, reps):
    import time as _time

    import jax

    out = f(*dev_args)
    jax.block_until_ready(out)
    ts = []
    for _ in range(reps):
        t0 = _time.perf_counter_ns()
        out = f(*dev_args)
        jax.block_until_ready(out)
        ts.append(_time.perf_counter_ns() - t0)
    ts.sort()
    return ts[len(ts) // 2], ts


def _build_null_program():
    nc = bacc.Bacc("TRN2", target_bir_lowering=False, debug=False, num_devices=NC)
    din = nc.dram_tensor("nullin", [128, 1], F32, kind="ExternalInput")
    dout = nc.dram_tensor("nullout", [128, 1], F32, kind="ExternalOutput")
    with tile.TileContext(nc) as tc, tc.tile_pool(name="np_", bufs=1) as p:
        t_ = p.tile([128, 1], F32, name="t_")
        nc.sync.dma_start(t_[:], din[:])
        nc.sync.dma_start(dout[:], t_[:])
    if not nc.is_finalized():
        nc.finalize()
    return nc


def measure_exec_ns(reps=30):
    global LAST_EXEC_NS
    nc = _TIME_STATE["nc"]
    in_maps = _TIME_STATE["in_maps"]
    f, dev_args = _make_pjrt_fn(nc, in_maps)
    t_full, ts_full = _time_fn(f, dev_args, reps)
    nc0 = _build_null_program()
    null_maps = [{"nullin": np.zeros((128, 1), np.float32)} for _ in range(NC)]
    f0, dev_args0 = _make_pjrt_fn(nc0, null_maps)
    t_null, ts_null = _time_fn(f0, dev_args0, reps)
    LAST_EXEC_NS = max(t_full - t_null, 0)
    return {
        "exec_ns": LAST_EXEC_NS,
        "full_median_ns": t_full,
        "null_median_ns": t_null,
        "full_min_ns": ts_full[0],
        "null_min_ns": ts_null[0],
    }
